# revision 38
# baseline (speedup 1.0000x reference)
"""Mixtral-style MoE (B=4, S=2048, H=2048, I=5632, E=8, top-2, integer softmax)
on 8 Trainium2 NeuronCores.

Strategy: expert-parallel. Routing (integer softmax + top-2 select) is computed
exactly on device; per-expert token sets are gathered to a fixed capacity C and
each core runs one expert's SwiGLU FFN over its gathered tokens in bf16 on the
PE array (PSUM accumulation in fp32). Host scatter-adds the weighted per-expert
outputs.

Self-contained: hardcodes all shapes; only needs the machine-level concourse /
jax environment.
"""
import os
import sys

if "/opt/trn_rl_repo" not in sys.path:
    sys.path.insert(0, "/opt/trn_rl_repo")

import numpy as np
import ml_dtypes

import concourse.bacc as bacc
import concourse.mybir as mybir
from concourse import tile
from concourse import bass_utils
from concourse.bass import broadcast_tensor_aps

# problem shapes
B, S, H, I, E = 4, 2048, 2048, 5632, 8
T = B * S                      # 8192 tokens
TOP_K = 2
Q_IN, LUT_MIN, Q_OUT = 128, -1024, 1 << 16

P = 128                        # partitions
KT = H // P                    # 16 contraction tiles for H
IT = I // P                    # 44 i-tiles
HT = H // P                    # 16 output tiles

BF16 = ml_dtypes.bfloat16

_EXP_LUT_CACHE = None


def _exp_lut():
    """Q16 exp LUT, computed with jax exactly as the reference does (jnp.exp
    differs from np.exp in the last ulp for ~half the entries, which shifts
    the int32 truncation)."""
    global _EXP_LUT_CACHE
    if _EXP_LUT_CACHE is None:
        import jax.numpy as jnp
        _EXP_LUT_CACHE = np.asarray(
            (jnp.exp(jnp.arange(LUT_MIN, 1, dtype=jnp.float32) / Q_IN) * Q_OUT
             ).astype(jnp.int32)
        )
    return _EXP_LUT_CACHE


def _route(x2d, w_gate):
    """Host replication of the reference integer-softmax top-2 routing
    (used only for debugging; the kernel routes on device)."""
    lg = (x2d.astype(np.float64) @ w_gate.T.astype(np.float64)).astype(np.float32)
    li = np.rint(lg * np.float32(128.0)).astype(np.int32)
    shifted = np.clip(li - li.max(axis=-1, keepdims=True), LUT_MIN, None)
    ev = _exp_lut()[shifted - LUT_MIN]                       # [T, E] int32
    gt = ev[:, None, :] > ev[:, :, None]                     # [T, e, j]
    eq = ev[:, None, :] == ev[:, :, None]
    jlt = np.arange(E)[None, None, :] < np.arange(E)[None, :, None]
    cnt = (gt | (eq & jlt)).sum(-1)
    sel = cnt < TOP_K
    evf = ev.astype(np.float32)
    den = (evf * sel).sum(-1, keepdims=True)
    wts = np.where(sel, evf / den, np.float32(0.0)).astype(np.float32)
    return sel, wts


TL = T // 8                    # tokens routed per core in phase 1
TT = TL // P
f32 = mybir.dt.float32
bf16 = mybir.dt.bfloat16
i32 = mybir.dt.int32
u16 = mybir.dt.uint16
MAGIC = 12582912.0             # 1.5 * 2^23: fp32 integer-round magic
_ROUTE_CACHE = []


def _build_route():
    """Phase 1: replicated gate, tokens sharded. Each core computes the fp32
    router matmul for T/8=1024 tokens exactly as the reference does on the PE
    (x-block stationary, 128-chunk PSUM accumulation in kt order) and rounds
    logits*128 half-even via the 1.5*2^23 magic constant. Output is the
    biased integer li' = round(logits*128) + 2^23*1.5 as fp32 [1024, E];
    everything downstream (clip, exp LUT, top-2, renorm) is exact integer /
    IEEE-fp32 arithmetic replicated on the host."""
    if _ROUTE_CACHE:
        return _ROUTE_CACHE[0]
    nc = bacc.Bacc("TRN2", target_bir_lowering=False, debug=False, num_devices=8)
    # xr pre-interleaved on host: xr[p, kt*TL + t] = x[token t, h=kt*128+p],
    # so each per-kt chunk DMA is a fully contiguous 4KB/partition line
    xr_d = nc.dram_tensor("xr", [P, KT * TL], f32, kind="ExternalInput").ap()
    # wg pre-interleaved on host: wg[p, kt*E + e] = w_gate[e, kt*128+p], so
    # the load is one contiguous 512B/partition line instead of 2048 32-byte
    # descriptors (which took ~13us and stalled every real matmul)
    wg_d = nc.dram_tensor("wg", [P, KT * E], f32, kind="ExternalInput").ap()
    # output kept in SBUF layout (contiguous per partition); host
    # de-interleaves li[tt*128+p, e] = out[p, tt*E+e]
    li_d = nc.dram_tensor("li", [P, TT * E], f32, kind="ExternalOutput").ap()

    with tile.TileContext(nc) as tc:
        with (
            tc.tile_pool(name="sb", bufs=1) as sb,
            tc.tile_pool(name="xrp", bufs=KT) as xrp,
            tc.tile_pool(name="ps", bufs=1, space="PSUM") as ps,
        ):
            # ~5us of dummy wide matmuls while the first xr chunk loads: HAM
            # un-throttles the PE clock (1.2 -> 2.4 GHz) before the real
            # LDW-bound router matmuls start; shares bank lg0, released
            # before the real accumulation begins
            wu_sb = sb.tile([P, 512], bf16, tag="wu_sb")
            nc.vector.memset(wu_sb[:], 0.0)
            wu_ps = ps.tile([P, 512], f32, tag="lg0")
            for r in range(12):
                nc.tensor.matmul(
                    wu_ps[:], wu_sb[:, :P], wu_sb[:],
                    start=(r == 0), stop=(r == 11),
                )

            wg_t = sb.tile([P, KT * E], f32, tag="wg")
            nc.scalar.dma_start(wg_t[:], wg_d[:])
            wg_v = wg_t[:].rearrange("p (kt e) -> p kt e", kt=KT)

            # activations arrive in contiguous per-kt chunks (~1.5us each);
            # the matmul loop runs kt-outer with all TT accumulators live in
            # PSUM, so each chunk is consumed as it lands. Per-(tt) the
            # accumulation order stays kt-ascending -> bit-exact logits.
            xr_tiles = []
            for kt in range(KT):
                xr_t = xrp.tile([P, TL], f32, tag="xr")
                nc.sync.dma_start(xr_t[:], xr_d[:, kt * TL:(kt + 1) * TL])
                xr_tiles.append(xr_t)

            li_t = sb.tile([P, TT * E], f32, tag="li")
            li_v = li_t[:].rearrange("p (tt e) -> p tt e", tt=TT)

            lg_tiles = []
            for tt in range(TT):
                lg_tt = ps.tile([P, E], f32, tag=f"lg{tt}")
                lg_tiles.append(lg_tt)
            for kt in range(KT):
                for tt in range(TT):
                    nc.tensor.matmul(
                        lg_tiles[tt][:],
                        xr_tiles[kt][:, tt * P:(tt + 1) * P],
                        wg_v[:, kt, :],
                        start=(kt == 0), stop=(kt == KT - 1),
                    )
            for tt in range(TT):
                # li' = round_half_even(logits*128) + MAGIC: *128 exact
                # (power of two), +2^23 rounds half-even
                nc.vector.tensor_scalar(
                    li_v[:, tt, :], lg_tiles[tt][:], 128.0, MAGIC,
                    mybir.AluOpType.mult, mybir.AluOpType.add,
                )

            nc.sync.dma_start(li_d[:], li_t[:])
    nc.compile()
    _ROUTE_CACHE.append(nc)
    return nc


def _weights_from_li(li):
    """Exact host replication of the reference integer softmax + top-2 given
    the integer router logits li = round(logits*128) [T, E] int32.

    Selection is decided purely by integer comparisons (distinct int ev
    always produce distinct fp32 routing weights since ev < 2^23, and ties
    break by lower expert index, matching jax.lax.top_k); weights are IEEE
    fp32 divisions identical to the reference's elementwise ops."""
    shifted = np.clip(li - li.max(axis=-1, keepdims=True), LUT_MIN, None)
    ev = _exp_lut()[shifted - LUT_MIN]                       # [T, E] int32
    # rank rule == jax.lax.top_k on ev/evsum (ties by lower index)
    gt = ev[:, None, :] > ev[:, :, None]                     # [T, e, j]
    eq = ev[:, None, :] == ev[:, :, None]
    jlt = np.arange(E)[None, None, :] < np.arange(E)[None, :, None]
    cnt = (gt | (eq & jlt)).sum(-1)
    sel = cnt < TOP_K
    evsum = np.clip(ev.sum(-1, keepdims=True), 1, None)      # int32, <= 2^20
    rw = ev.astype(np.float32) / evsum.astype(np.float32)    # [T, E] fp32
    den = (rw * sel).sum(-1, keepdims=True, dtype=np.float32)
    wts = np.where(sel, rw / den, np.float32(0.0)).astype(np.float32)
    return sel, wts


_BUILD_CACHE = {}


def _ffn_blocks(C):
    """Split capacity C into clean 512-wide blocks, with any remainder merged
    into the last block (so every block streams its weights exactly once and
    no DMA-bound small tail block exists). Max block width 512+448=960 spans
    2 PSUM banks; gu 2-deep + y 2-deep fit in the 8 banks."""
    nfull, rem = divmod(C, 512)
    if nfull == 0:
        widths = [C]
    elif rem == 0:
        widths = [512] * nfull
    else:
        widths = [512] * (nfull - 1) + [512 + rem]
    blocks = []
    t0 = 0
    for w in widths:
        blocks.append((t0, w))
        t0 += w
    return blocks


def _build_ffn(C):
    """Bass program: one expert's SwiGLU FFN over C gathered tokens, bf16.

    yt[h, t] = wv[t] * ( (silu(x @ w1.T) * (x @ w3.T)) @ w2.T )[t, h]

    Layouts (host-prepared, all bf16 except wv):
      xt  [H, C]            x gathered+transposed
      w13 [IT, 128, 2H]     w13[it, p, kt*128+i]     = w1[it*128+i, kt*128+p]
                            w13[it, p, H + kt*128+i] = w3[it*128+i, kt*128+p]
      w2p [HT, 128, I]      w2p[ht, p, it*128+h]     = w2[ht*128+h, it*128+p]
      wv  [128, C] f32      combine weights replicated across partitions
      yt  [H, C]            output (transposed), bf16
    """
    if C in _BUILD_CACHE:
        return _BUILD_CACHE[C]

    blocks = _ffn_blocks(C)

    nc = bacc.Bacc("TRN2", target_bir_lowering=False, debug=False, num_devices=8)
    xt_d = nc.dram_tensor("xt", [H, C], bf16, kind="ExternalInput").ap()
    w13_d = nc.dram_tensor("w13p", [IT, P, 2 * H], bf16, kind="ExternalInput").ap()
    w2_d = nc.dram_tensor("w2p", [HT, P, I], bf16, kind="ExternalInput").ap()
    wv_d = nc.dram_tensor("wv", [P, C], f32, kind="ExternalInput").ap()
    yt_d = nc.dram_tensor("yt", [H, C], bf16, kind="ExternalOutput").ap()

    with tile.TileContext(nc) as tc:
        with (
            tc.tile_pool(name="wv", bufs=2) as wv_pool,
            tc.tile_pool(name="xt", bufs=2 * KT) as xt_pool,
            tc.tile_pool(name="w13", bufs=8) as w13_pool,
            tc.tile_pool(name="w2", bufs=3) as w2_pool,
            tc.tile_pool(name="h", bufs=1) as h_pool,
            tc.tile_pool(name="silu", bufs=2) as silu_pool,
            tc.tile_pool(name="ysb", bufs=2) as ysb_pool,
            tc.tile_pool(name="gu_ps", bufs=2, space="PSUM") as gu_pool,
            tc.tile_pool(name="y_ps", bufs=2, space="PSUM") as y_pool,
        ):
            # ~5us of dummy matmuls while the first weight/activation DMAs
            # land: warms the HAM clock gate to 2.4 GHz before real work
            wu_sb = silu_pool.tile([P, 512], bf16, tag="wu_sb")
            nc.vector.memset(wu_sb[:], 0.0)
            wu_ps = gu_pool.tile([P, 512], f32, tag="gu")
            for r in range(12):
                nc.tensor.matmul(
                    wu_ps[:], wu_sb[:, :P], wu_sb[:],
                    start=(r == 0), stop=(r == 11),
                )

            for (tok0, W) in blocks:
                ts = slice(tok0, tok0 + W)
                # sub-ranges of W for <=512-wide matmul moving operands
                subs = [(s0, min(512, W - s0)) for s0 in range(0, W, 512)]

                # activations for this token block, one tile per h-chunk so
                # the first matmul only waits for the first ~W*256B transfer
                xt_kt = []
                for kt in range(KT):
                    x1 = xt_pool.tile([P, W], bf16, tag="xt")
                    nc.scalar.dma_start(x1[:], xt_d[kt * P:(kt + 1) * P, ts])
                    xt_kt.append(x1)

                h_t = h_pool.tile([P, IT * W], bf16, tag="h")
                h_v = h_t[:].rearrange("p (it t) -> p it t", it=IT)

                # ---- phase A: h[i, t] = silu(g) * u over all I tiles ----
                for it in range(IT):
                    w13_t = w13_pool.tile([P, 2 * H], bf16, tag="w13")
                    nc.sync.dma_start(w13_t[:], w13_d[it])

                    g_ps = gu_pool.tile([P, W], f32, tag="gu")
                    u_ps = gu_pool.tile([P, W], f32, tag="gu")
                    for half, ps_t in ((0, g_ps), (1, u_ps)):
                        for kt in range(KT):
                            wsl = w13_t[:, half * H + kt * P:half * H + (kt + 1) * P]
                            for (s0, sw) in subs:
                                nc.tensor.matmul(
                                    ps_t[:, s0:s0 + sw],
                                    wsl,
                                    xt_kt[kt][:, s0:s0 + sw],
                                    start=(kt == 0), stop=(kt == KT - 1),
                                )
                    sg = silu_pool.tile([P, W], f32, tag="silu")
                    nc.scalar.activation(
                        sg[:], g_ps[:], mybir.ActivationFunctionType.Silu
                    )
                    nc.vector.tensor_tensor(
                        h_v[:, it, :], sg[:], u_ps[:], op=mybir.AluOpType.mult
                    )

                # ---- phase B: yt[h, t] = wv[t] * (w2 @ h) ----
                wv_t = wv_pool.tile([P, W], f32, tag="wv")
                nc.scalar.dma_start(wv_t[:], wv_d[:, ts])
                for ht in range(HT):
                    w2_t = w2_pool.tile([P, I], bf16, tag="w2")
                    nc.scalar.dma_start(w2_t[:], w2_d[ht])
                    y_ps = y_pool.tile([P, W], f32, tag="y")
                    for it in range(IT):
                        wsl = w2_t[:, it * P:(it + 1) * P]
                        for (s0, sw) in subs:
                            nc.tensor.matmul(
                                y_ps[:, s0:s0 + sw],
                                wsl,
                                h_v[:, it, s0:s0 + sw],
                                start=(it == 0), stop=(it == IT - 1),
                            )
                    y_sb = ysb_pool.tile([P, W], bf16, tag="ysb")
                    nc.vector.tensor_tensor(
                        y_sb[:], y_ps[:], wv_t[:], op=mybir.AluOpType.mult
                    )
                    nc.sync.dma_start(yt_d[ht * P:(ht + 1) * P, ts], y_sb[:])

    nc.compile()
    _BUILD_CACHE[C] = nc
    return nc


def _prep_weights(w1, w2, w3):
    """Pretile per-expert weights into SBUF-friendly bf16 layouts:
      w13[e][it, p, kt*128+i]     = w1[e][it*128+i, kt*128+p]   ([IT, 128, 2H])
      w13[e][it, p, H + kt*128+i] = w3[e][it*128+i, kt*128+p]
      w2p[e][ht, p, it*128+h]     = w2[e][ht*128+h, it*128+p]   ([HT, 128, I])
    """
    w13p = np.empty((E, IT, P, 2 * H), BF16)
    w13p[:, :, :, :H] = w1.reshape(E, IT, P, KT, P).transpose(0, 1, 4, 3, 2).reshape(
        E, IT, P, H).astype(BF16)
    w13p[:, :, :, H:] = w3.reshape(E, IT, P, KT, P).transpose(0, 1, 4, 3, 2).reshape(
        E, IT, P, H).astype(BF16)
    w2p = np.ascontiguousarray(
        w2.reshape(E, HT, P, IT, P).transpose(0, 1, 4, 3, 2)
    ).reshape(E, HT, P, I).astype(BF16)
    return w13p, w2p


def kernel(x, w_gate, w1, w2, w3):
    x = np.asarray(x, dtype=np.float32)
    w_gate = np.asarray(w_gate, dtype=np.float32)
    w1 = np.asarray(w1, dtype=np.float32)
    w2 = np.asarray(w2, dtype=np.float32)
    w3 = np.asarray(w3, dtype=np.float32)

    x2d = x.reshape(T, H)
    trace = bool(int(os.environ.get("BASS_MOE_TRACE", "0")))

    # ---- phase 1: routing on device (replicated gate, tokens sharded) ----
    nc1 = _build_route()
    wgT = np.ascontiguousarray(
        w_gate.T.reshape(KT, P, E).transpose(1, 0, 2)).reshape(P, KT * E)
    in1 = [
        {"xr": np.ascontiguousarray(
            x2d[c * TL:(c + 1) * TL].reshape(TL, KT, P).transpose(2, 1, 0)
        ).reshape(P, KT * TL),
         "wg": wgT}
        for c in range(8)
    ]
    res1 = bass_utils.run_bass_kernel_spmd(
        nc1, in1, core_ids=list(range(8)), trace=trace
    )
    li_b = np.concatenate([
        res1.results[c]["li"].reshape(P, TT, E).transpose(1, 0, 2).reshape(TL, E)
        for c in range(8)
    ], 0)
    li = (li_b - np.float32(MAGIC)).astype(np.int32)
    sel, wts = _weights_from_li(li)
    counts = sel.sum(0)
    C = max(512, -(-int(counts.max()) // 16) * 16)

    w13p, w2p = _prep_weights(w1, w2, w3)
    x2d_bf = x2d.astype(BF16)

    idxs, in_maps = [], []
    for e in range(E):
        idx = np.nonzero(sel[:, e])[0]
        idxs.append(idx)
        xsel = np.zeros((C, H), BF16)
        xsel[:len(idx)] = x2d_bf[idx]
        wv = np.zeros(C, np.float32)
        wv[:len(idx)] = wts[idx, e]
        in_maps.append({
            "xt": np.ascontiguousarray(xsel.T),
            "w13p": w13p[e],
            "w2p": w2p[e],
            "wv": np.broadcast_to(wv, (P, C)).copy(),
        })

    nc = _build_ffn(C)
    res = bass_utils.run_bass_kernel_spmd(
        nc, in_maps, core_ids=list(range(8)), trace=trace
    )
    if trace:
        kernel.route_ns = res1.exec_time_ns
        kernel.ffn_ns = res.exec_time_ns
        kernel.last_exec_time_ns = (res1.exec_time_ns or 0) + (res.exec_time_ns or 0)
        kernel.route_trace = getattr(res1, "instructions_and_trace", None)
        kernel.ffn_trace = getattr(res, "instructions_and_trace", None)

    out2d = np.zeros((T, H), np.float32)
    for e in range(E):
        idx = idxs[e]
        out2d[idx] += res.results[e]["yt"].T[:len(idx)].astype(np.float32)
    return out2d.reshape(B, S, H)


kernel.last_exec_time_ns = None
kernel.route_ns = None
kernel.ffn_ns = None


# revision 40
# speedup vs baseline: 1.1659x; 1.1659x over previous
"""Mixtral-style MoE (B=4, S=2048, H=2048, I=5632, E=8, top-2, integer softmax)
on 8 Trainium2 NeuronCores.

Strategy: expert-parallel, two launches.
1) Routing: each core computes the fp32 router matmul for T/8 tokens on the PE
   exactly as the reference does (x stationary, kt-ascending accumulation) and
   returns the rounded integer logits; the integer softmax + top-2 select +
   renormalization are replicated bit-exactly on the host (selection depends
   only on integer comparisons).
2) FFN: per-expert token sets are gathered to a fixed capacity C and each core
   runs one expert's SwiGLU FFN over its gathered tokens in bf16 on the PE
   array (PSUM accumulation in fp32). Host scatter-adds the weighted
   per-expert outputs.

Self-contained: hardcodes all shapes; only needs the machine-level concourse /
jax environment.
"""
import os
import sys

if "/opt/trn_rl_repo" not in sys.path:
    sys.path.insert(0, "/opt/trn_rl_repo")

import numpy as np
import ml_dtypes

import concourse.bacc as bacc
import concourse.mybir as mybir
from concourse import tile
from concourse import bass_utils

# problem shapes
B, S, H, I, E = 4, 2048, 2048, 5632, 8
T = B * S                      # 8192 tokens
TOP_K = 2
Q_IN, LUT_MIN, Q_OUT = 128, -1024, 1 << 16

P = 128                        # partitions
KT = H // P                    # 16 contraction tiles for H
IT = I // P                    # 44 i-tiles
HT = H // P                    # 16 output tiles

BF16 = ml_dtypes.bfloat16

_EXP_LUT_CACHE = None


def _exp_lut():
    """Q16 exp LUT, computed with jax exactly as the reference does (jnp.exp
    differs from np.exp in the last ulp for ~half the entries, which shifts
    the int32 truncation)."""
    global _EXP_LUT_CACHE
    if _EXP_LUT_CACHE is None:
        import jax.numpy as jnp
        _EXP_LUT_CACHE = np.asarray(
            (jnp.exp(jnp.arange(LUT_MIN, 1, dtype=jnp.float32) / Q_IN) * Q_OUT
             ).astype(jnp.int32)
        )
    return _EXP_LUT_CACHE


def _route(x2d, w_gate):
    """Host replication of the reference integer-softmax top-2 routing
    (used only for debugging; the kernel routes on device)."""
    lg = (x2d.astype(np.float64) @ w_gate.T.astype(np.float64)).astype(np.float32)
    li = np.rint(lg * np.float32(128.0)).astype(np.int32)
    shifted = np.clip(li - li.max(axis=-1, keepdims=True), LUT_MIN, None)
    ev = _exp_lut()[shifted - LUT_MIN]                       # [T, E] int32
    gt = ev[:, None, :] > ev[:, :, None]                     # [T, e, j]
    eq = ev[:, None, :] == ev[:, :, None]
    jlt = np.arange(E)[None, None, :] < np.arange(E)[None, :, None]
    cnt = (gt | (eq & jlt)).sum(-1)
    sel = cnt < TOP_K
    evf = ev.astype(np.float32)
    den = (evf * sel).sum(-1, keepdims=True)
    wts = np.where(sel, evf / den, np.float32(0.0)).astype(np.float32)
    return sel, wts


TL = T // 8                    # tokens routed per core in phase 1
TT = TL // P
f32 = mybir.dt.float32
bf16 = mybir.dt.bfloat16
MAGIC = 12582912.0             # 1.5 * 2^23: fp32 integer-round magic
_ROUTE_CACHE = []


def _build_route():
    """Phase 1: replicated gate, tokens sharded. Each core computes the fp32
    router matmul for T/8=1024 tokens exactly as the reference does on the PE
    (x-block stationary, 128-chunk PSUM accumulation in kt order) and rounds
    logits*128 half-even via the 1.5*2^23 magic constant. Output is the
    biased integer li' = round(logits*128) + 2^23*1.5 as fp32 [1024, E];
    everything downstream (clip, exp LUT, top-2, renorm) is exact integer /
    IEEE-fp32 arithmetic replicated on the host."""
    if _ROUTE_CACHE:
        return _ROUTE_CACHE[0]
    nc = bacc.Bacc("TRN2", target_bir_lowering=False, debug=False, num_devices=8)
    # xr pre-interleaved on host: xr[p, kt*TL + t] = x[token t, h=kt*128+p],
    # so each per-kt chunk DMA is a fully contiguous 4KB/partition line
    xr_d = nc.dram_tensor("xr", [P, KT * TL], f32, kind="ExternalInput").ap()
    # wg pre-interleaved on host: wg[p, kt*E + e] = w_gate[e, kt*128+p], so
    # the load is one contiguous 512B/partition line instead of 2048 32-byte
    # descriptors (which took ~13us and stalled every real matmul)
    wg_d = nc.dram_tensor("wg", [P, KT * E], f32, kind="ExternalInput").ap()
    # output kept in SBUF layout (contiguous per partition); host
    # de-interleaves li[tt*128+p, e] = out[p, tt*E+e]
    li_d = nc.dram_tensor("li", [P, TT * E], f32, kind="ExternalOutput").ap()

    with tile.TileContext(nc) as tc:
        with (
            tc.tile_pool(name="sb", bufs=1) as sb,
            tc.tile_pool(name="xrp", bufs=KT) as xrp,
            tc.tile_pool(name="ps", bufs=1, space="PSUM") as ps,
        ):
            # ~5us of dummy wide matmuls while the first xr chunk loads: HAM
            # un-throttles the PE clock (1.2 -> 2.4 GHz) before the real
            # LDW-bound router matmuls start; shares bank lg0, released
            # before the real accumulation begins
            wu_sb = sb.tile([P, 512], bf16, tag="wu_sb")
            nc.vector.memset(wu_sb[:], 0.0)
            wu_ps = ps.tile([P, 512], f32, tag="lg0")
            for r in range(12):
                nc.tensor.matmul(
                    wu_ps[:], wu_sb[:, :P], wu_sb[:],
                    start=(r == 0), stop=(r == 11),
                )

            wg_t = sb.tile([P, KT * E], f32, tag="wg")
            nc.scalar.dma_start(wg_t[:], wg_d[:])
            wg_v = wg_t[:].rearrange("p (kt e) -> p kt e", kt=KT)

            # activations arrive in contiguous per-kt chunks (~1.5us each);
            # the matmul loop runs kt-outer with all TT accumulators live in
            # PSUM, so each chunk is consumed as it lands. Per-(tt) the
            # accumulation order stays kt-ascending -> bit-exact logits.
            xr_tiles = []
            for kt in range(KT):
                xr_t = xrp.tile([P, TL], f32, tag="xr")
                nc.sync.dma_start(xr_t[:], xr_d[:, kt * TL:(kt + 1) * TL])
                xr_tiles.append(xr_t)

            li_t = sb.tile([P, TT * E], f32, tag="li")
            li_v = li_t[:].rearrange("p (tt e) -> p tt e", tt=TT)

            lg_tiles = []
            for tt in range(TT):
                lg_tt = ps.tile([P, E], f32, tag=f"lg{tt}")
                lg_tiles.append(lg_tt)
            for kt in range(KT):
                for tt in range(TT):
                    nc.tensor.matmul(
                        lg_tiles[tt][:],
                        xr_tiles[kt][:, tt * P:(tt + 1) * P],
                        wg_v[:, kt, :],
                        start=(kt == 0), stop=(kt == KT - 1),
                    )
            for tt in range(TT):
                # li' = round_half_even(logits*128) + MAGIC: *128 exact
                # (power of two), +2^23 rounds half-even
                nc.vector.tensor_scalar(
                    li_v[:, tt, :], lg_tiles[tt][:], 128.0, MAGIC,
                    mybir.AluOpType.mult, mybir.AluOpType.add,
                )

            nc.sync.dma_start(li_d[:], li_t[:])
    nc.compile()
    _ROUTE_CACHE.append(nc)
    return nc


def _weights_from_li(li):
    """Exact host replication of the reference integer softmax + top-2 given
    the integer router logits li = round(logits*128) [T, E] int32.

    Selection is decided purely by integer comparisons (distinct int ev
    always produce distinct fp32 routing weights since ev < 2^23, and ties
    break by lower expert index, matching jax.lax.top_k); weights are IEEE
    fp32 divisions identical to the reference's elementwise ops."""
    shifted = np.clip(li - li.max(axis=-1, keepdims=True), LUT_MIN, None)
    ev = _exp_lut()[shifted - LUT_MIN]                       # [T, E] int32
    # rank rule == jax.lax.top_k on ev/evsum (ties by lower index)
    gt = ev[:, None, :] > ev[:, :, None]                     # [T, e, j]
    eq = ev[:, None, :] == ev[:, :, None]
    jlt = np.arange(E)[None, None, :] < np.arange(E)[None, :, None]
    cnt = (gt | (eq & jlt)).sum(-1)
    sel = cnt < TOP_K
    evsum = np.clip(ev.sum(-1, keepdims=True), 1, None)      # int32, <= 2^20
    rw = ev.astype(np.float32) / evsum.astype(np.float32)    # [T, E] fp32
    den = (rw * sel).sum(-1, keepdims=True, dtype=np.float32)
    wts = np.where(sel, rw / den, np.float32(0.0)).astype(np.float32)
    return sel, wts


_BUILD_CACHE = {}


def _ffn_blocks(C):
    """Split capacity C into clean 512-wide blocks, with any remainder merged
    into the last block (so every block streams its weights exactly once and
    no DMA-bound small tail block exists). Max block width 512+448=960 spans
    2 PSUM banks; gu 2-deep + y 2-deep fit in the 8 banks."""
    nfull, rem = divmod(C, 512)
    if nfull == 0:
        widths = [C]
    elif rem == 0:
        widths = [512] * nfull
    else:
        widths = [512] * (nfull - 1) + [512 + rem]
    blocks = []
    t0 = 0
    for w in widths:
        blocks.append((t0, w))
        t0 += w
    return blocks


def _build_ffn(C):
    """Bass program: one expert's SwiGLU FFN over C gathered tokens, bf16.

    yt[h, t] = wv[t] * ( (silu(x @ w1.T) * (x @ w3.T)) @ w2.T )[t, h]

    Layouts (host-prepared, all bf16 except wv):
      xt  [H, C]            x gathered+transposed
      w13 [IT, 128, 2H]     w13[it, p, kt*128+i]     = w1[it*128+i, kt*128+p]
                            w13[it, p, H + kt*128+i] = w3[it*128+i, kt*128+p]
      w2p [HT, 128, I]      w2p[ht, p, it*128+h]     = w2[ht*128+h, it*128+p]
      wv  [128, C] f32      combine weights replicated across partitions
      yt  [H, C]            output (transposed), bf16
    """
    if C in _BUILD_CACHE:
        return _BUILD_CACHE[C]

    blocks = _ffn_blocks(C)

    nc = bacc.Bacc("TRN2", target_bir_lowering=False, debug=False, num_devices=8)
    xt_d = nc.dram_tensor("xt", [H, C], bf16, kind="ExternalInput").ap()
    w13_d = nc.dram_tensor("w13p", [IT, P, 2 * H], bf16, kind="ExternalInput").ap()
    w2_d = nc.dram_tensor("w2p", [HT, P, I], bf16, kind="ExternalInput").ap()
    wv_d = nc.dram_tensor("wv", [P, C], f32, kind="ExternalInput").ap()
    yt_d = nc.dram_tensor("yt", [H, C], bf16, kind="ExternalOutput").ap()

    with tile.TileContext(nc) as tc:
        with (
            tc.tile_pool(name="wv", bufs=2) as wv_pool,
            tc.tile_pool(name="xt", bufs=2 * KT) as xt_pool,
            tc.tile_pool(name="w13", bufs=8) as w13_pool,
            tc.tile_pool(name="w2", bufs=3) as w2_pool,
            tc.tile_pool(name="h", bufs=1) as h_pool,
            tc.tile_pool(name="silu", bufs=2) as silu_pool,
            tc.tile_pool(name="ysb", bufs=2) as ysb_pool,
            tc.tile_pool(name="gu_ps", bufs=2, space="PSUM") as gu_pool,
            tc.tile_pool(name="y_ps", bufs=2, space="PSUM") as y_pool,
        ):
            # ~5us of dummy matmuls while the first weight/activation DMAs
            # land: warms the HAM clock gate to 2.4 GHz before real work
            wu_sb = silu_pool.tile([P, 512], bf16, tag="wu_sb")
            nc.vector.memset(wu_sb[:], 0.0)
            wu_ps = gu_pool.tile([P, 512], f32, tag="gu")
            for r in range(12):
                nc.tensor.matmul(
                    wu_ps[:], wu_sb[:, :P], wu_sb[:],
                    start=(r == 0), stop=(r == 11),
                )

            for (tok0, W) in blocks:
                ts = slice(tok0, tok0 + W)
                # sub-ranges of W for <=512-wide matmul moving operands
                subs = [(s0, min(512, W - s0)) for s0 in range(0, W, 512)]

                # activations for this token block, one tile per h-chunk so
                # the first matmul only waits for the first ~W*256B transfer
                xt_kt = []
                for kt in range(KT):
                    x1 = xt_pool.tile([P, W], bf16, tag="xt")
                    nc.scalar.dma_start(x1[:], xt_d[kt * P:(kt + 1) * P, ts])
                    xt_kt.append(x1)

                h_t = h_pool.tile([P, IT * W], bf16, tag="h")
                h_v = h_t[:].rearrange("p (it t) -> p it t", it=IT)

                # ---- phase A: h[i, t] = silu(g) * u over all I tiles ----
                for it in range(IT):
                    w13_t = w13_pool.tile([P, 2 * H], bf16, tag="w13")
                    nc.sync.dma_start(w13_t[:], w13_d[it])

                    g_ps = gu_pool.tile([P, W], f32, tag="gu")
                    u_ps = gu_pool.tile([P, W], f32, tag="gu")
                    for half, ps_t in ((0, g_ps), (1, u_ps)):
                        for kt in range(KT):
                            wsl = w13_t[:, half * H + kt * P:half * H + (kt + 1) * P]
                            for (s0, sw) in subs:
                                nc.tensor.matmul(
                                    ps_t[:, s0:s0 + sw],
                                    wsl,
                                    xt_kt[kt][:, s0:s0 + sw],
                                    start=(kt == 0), stop=(kt == KT - 1),
                                )
                    sg = silu_pool.tile([P, W], f32, tag="silu")
                    nc.scalar.activation(
                        sg[:], g_ps[:], mybir.ActivationFunctionType.Silu
                    )
                    nc.vector.tensor_tensor(
                        h_v[:, it, :], sg[:], u_ps[:], op=mybir.AluOpType.mult
                    )

                # ---- phase B: yt[h, t] = wv[t] * (w2 @ h) ----
                wv_t = wv_pool.tile([P, W], f32, tag="wv")
                nc.scalar.dma_start(wv_t[:], wv_d[:, ts])
                for ht in range(HT):
                    w2_t = w2_pool.tile([P, I], bf16, tag="w2")
                    nc.scalar.dma_start(w2_t[:], w2_d[ht])
                    y_ps = y_pool.tile([P, W], f32, tag="y")
                    for it in range(IT):
                        wsl = w2_t[:, it * P:(it + 1) * P]
                        for (s0, sw) in subs:
                            nc.tensor.matmul(
                                y_ps[:, s0:s0 + sw],
                                wsl,
                                h_v[:, it, s0:s0 + sw],
                                start=(it == 0), stop=(it == IT - 1),
                            )
                    y_sb = ysb_pool.tile([P, W], bf16, tag="ysb")
                    nc.vector.tensor_tensor(
                        y_sb[:], y_ps[:], wv_t[:], op=mybir.AluOpType.mult
                    )
                    nc.sync.dma_start(yt_d[ht * P:(ht + 1) * P, ts], y_sb[:])

    nc.compile()
    _BUILD_CACHE[C] = nc
    return nc


def _prep_weights(w1, w2, w3):
    """Pretile per-expert weights into SBUF-friendly bf16 layouts:
      w13[e][it, p, kt*128+i]     = w1[e][it*128+i, kt*128+p]   ([IT, 128, 2H])
      w13[e][it, p, H + kt*128+i] = w3[e][it*128+i, kt*128+p]
      w2p[e][ht, p, it*128+h]     = w2[e][ht*128+h, it*128+p]   ([HT, 128, I])
    """
    w13p = np.empty((E, IT, P, 2 * H), BF16)
    w13p[:, :, :, :H] = w1.reshape(E, IT, P, KT, P).transpose(0, 1, 4, 3, 2).reshape(
        E, IT, P, H).astype(BF16)
    w13p[:, :, :, H:] = w3.reshape(E, IT, P, KT, P).transpose(0, 1, 4, 3, 2).reshape(
        E, IT, P, H).astype(BF16)
    w2p = np.ascontiguousarray(
        w2.reshape(E, HT, P, IT, P).transpose(0, 1, 4, 3, 2)
    ).reshape(E, HT, P, I).astype(BF16)
    return w13p, w2p


def kernel(x, w_gate, w1, w2, w3):
    x = np.asarray(x, dtype=np.float32)
    w_gate = np.asarray(w_gate, dtype=np.float32)
    w1 = np.asarray(w1, dtype=np.float32)
    w2 = np.asarray(w2, dtype=np.float32)
    w3 = np.asarray(w3, dtype=np.float32)

    x2d = x.reshape(T, H)
    trace = bool(int(os.environ.get("BASS_MOE_TRACE", "0")))

    # ---- phase 1: routing on device (replicated gate, tokens sharded) ----
    nc1 = _build_route()
    wgT = np.ascontiguousarray(
        w_gate.T.reshape(KT, P, E).transpose(1, 0, 2)).reshape(P, KT * E)
    in1 = [
        {"xr": np.ascontiguousarray(
            x2d[c * TL:(c + 1) * TL].reshape(TL, KT, P).transpose(2, 1, 0)
        ).reshape(P, KT * TL),
         "wg": wgT}
        for c in range(8)
    ]
    res1 = bass_utils.run_bass_kernel_spmd(
        nc1, in1, core_ids=list(range(8)), trace=trace
    )
    li_b = np.concatenate([
        res1.results[c]["li"].reshape(P, TT, E).transpose(1, 0, 2).reshape(TL, E)
        for c in range(8)
    ], 0)
    li = (li_b - np.float32(MAGIC)).astype(np.int32)
    sel, wts = _weights_from_li(li)
    counts = sel.sum(0)
    C = max(512, -(-int(counts.max()) // 16) * 16)

    w13p, w2p = _prep_weights(w1, w2, w3)
    x2d_bf = x2d.astype(BF16)

    idxs, in_maps = [], []
    for e in range(E):
        idx = np.nonzero(sel[:, e])[0]
        idxs.append(idx)
        xsel = np.zeros((C, H), BF16)
        xsel[:len(idx)] = x2d_bf[idx]
        wv = np.zeros(C, np.float32)
        wv[:len(idx)] = wts[idx, e]
        in_maps.append({
            "xt": np.ascontiguousarray(xsel.T),
            "w13p": w13p[e],
            "w2p": w2p[e],
            "wv": np.broadcast_to(wv, (P, C)).copy(),
        })

    nc = _build_ffn(C)
    res = bass_utils.run_bass_kernel_spmd(
        nc, in_maps, core_ids=list(range(8)), trace=trace
    )
    if trace:
        kernel.route_ns = res1.exec_time_ns
        kernel.ffn_ns = res.exec_time_ns
        kernel.last_exec_time_ns = (res1.exec_time_ns or 0) + (res.exec_time_ns or 0)
        kernel.route_trace = getattr(res1, "instructions_and_trace", None)
        kernel.ffn_trace = getattr(res, "instructions_and_trace", None)

    out2d = np.zeros((T, H), np.float32)
    for e in range(E):
        idx = idxs[e]
        out2d[idx] += res.results[e]["yt"].T[:len(idx)].astype(np.float32)
    return out2d.reshape(B, S, H)


kernel.last_exec_time_ns = None
kernel.route_ns = None
kernel.ffn_ns = None
